# revision 45
# baseline (speedup 1.0000x reference)
"""DeepseekV2 MLA attention (T=2048, H=16) on 8 trn2 cores.

Sharding v3 (all-bf16, throttle-aware):
- Stage 1 (a-projections) sequence-sharded: each core computes q_c /
  kv latent / roped k_pe for its own 256 tokens.
- kv latent (+ roped k_pe) AllGather'd; q up-projection sequence-sharded
  (own tokens, ALL 16 heads, rms scale + neox rope folded pre-exchange),
  then an AllToAll redistributes q by head-pair (2 heads/core, full T).
- The neox rope partner (-x2,x1) is formed by a DVE partition shuffle of
  the single rope projection psum (sin table carries the sign), instead
  of projecting a second rotated weight copy: saves 1/4 of the q b-proj
  matmuls + weights, and 1 of 6 kv a-proj tiles.
- Everything on the PE is bf16 (measured: bf16@512-free == f32r speed,
  at half the SBUF/DMA traffic; the chip power-throttles under load so
  every byte moved costs PE speed too). PSUM accumulation stays f32.
- Softmax denominator off the PE: DVE accumulates exp tiles into l_acc
  and tree-reduces over partitions; only the 1/l broadcast uses a tiny
  f32r ones-matmul. Causal diag via 0/1 tri mask post-exp. No row-max
  (logits ~N(0,1)). o_proj partials summed on the host (y in bf16).
- No dummy collective: the TileContext prelude barrier absorbs the
  ~40us comm bring-up; the kv AllGather queues behind it harmlessly.
- DMA triggers spread by engine: sync = weight streams + h tiles,
  scalar = persists + readbacks, gpsimd = collectives + payload writes.
"""

import numpy as np

T = 2048
HID = 2048
H = 16
NC_ = 8
HLOC = H // NC_          # 2 heads per core
NP = H // HLOC           # 8 head-pairs
QL = 1536                # q lora
KVL = 512                # kv lora
DN = 128                 # nope dim
DR = 64                  # rope dim
DQK = DN + DR            # 192
DV = 128
EPS = 1e-6
SCALE = float(DQK) ** -0.5
P = 128
TC = T // NC_            # 256 tokens per core (stage-1 shard)
NKQ = QL // P            # 12
NKV = KVL // P           # 4
NKH = HID // P           # 16
QC = 512                 # attention q-chunk
NQC = T // QC
NKB = T // P             # key blocks

_CACHE = {}
LAST_RESULTS = None


def _split_multi_waits(nc, mybir):
    """Walrus embeds at most one sem/event wait per TPB instruction; hoist
    extra waits onto preceding same-engine NoOps (queue FIFO keeps order)."""
    n = 0
    for f in nc.m.functions:
        for bb in f.blocks:
            new = []
            for inst in bb.instructions:
                si = getattr(inst, "sync_info", None)
                if si is not None and len(si.on_wait) > 1:
                    waits = list(si.on_wait)
                    for i, wv in enumerate(waits[:-1]):
                        noop = mybir.InstNoOp(
                            name=f"{inst.name}-wsplit{i}",
                            engine=inst.engine,
                            ins=[],
                            outs=[],
                        )
                        noop.bass_nofuse = True
                        noop.sync_info = mybir.SyncInfo(on_wait=[wv], on_update=[])
                        new.append(noop)
                    inst.sync_info = mybir.SyncInfo(
                        on_wait=[waits[-1]], on_update=list(si.on_update)
                    )
                    n += 1
                new.append(inst)
            bb.instructions = new
    return n


def _build_program():
    import concourse.bass as bass
    import concourse.tile as tile
    from concourse import mybir

    f32 = mybir.dt.float32
    bf16 = mybir.dt.bfloat16
    f32r = mybir.dt.float32r
    AF = mybir.ActivationFunctionType
    GRP = [list(range(NC_))]

    nc = bass.Bass(num_devices=NC_)

    # ---- dram parameters (per-core values supplied by the host) ----
    hT_d = nc.declare_dram_parameter("hT", [P, NKH, TC], bf16, isOutput=False)
    wqa_d = nc.declare_dram_parameter("wqa", [P, NKQ, NKH, P], bf16, isOutput=False)
    # latent 512 | ropeA dup 128
    wkva_d = nc.declare_dram_parameter(
        "wkva", [P, NKV + 1, NKH, P], bf16, isOutput=False
    )
    # q b-projection for ALL head-pairs: [p, pair, mo, k, col]
    # mo: 0/1 = nope h0/h1, 2 = ropeA (h0|h1)  (ln folded)
    wqb_d = nc.declare_dram_parameter("wqb", [P, NP, 3, NKQ, P], bf16, isOutput=False)
    wkvbk_d = nc.declare_dram_parameter("wkvbk", [P, NKV, HLOC * DN], bf16, isOutput=False)
    wkvbv_d = nc.declare_dram_parameter("wkvbv", [P, NKV, HLOC * DV], bf16, isOutput=False)
    wo_d = nc.declare_dram_parameter("wo", [P, HLOC, HID], bf16, isOutput=False)
    cosl_d = nc.declare_dram_parameter("cosl", [P, TC], f32, isOutput=False)
    sinl_d = nc.declare_dram_parameter("sinl", [P, TC], f32, isOutput=False)
    # signed permutation forming the neox rope partner (-x2,x1) per 64-block
    permr_d = nc.declare_dram_parameter("permr", [P, P], bf16, isOutput=False)
    trimask_d = nc.declare_dram_parameter("trimask", [P, P], bf16, isOutput=False)
    y_d = nc.declare_dram_parameter("y", [T, HID], bf16, isOutput=True)

    # ---- dram bounce buffers for the collectives ----
    # kv payload: 4 latent tiles + 1 roped kpe (dup) tile, [p, m, t]
    kv_in = nc.dram_tensor("kv_in", [P, NKV + 1, TC], bf16)
    kv_out = nc.dram_tensor(
        "kv_out", [NC_, P, NKV + 1, TC], bf16, addr_space="Shared"
    )
    # q payload: per dst head-pair [p, mo(3), t]: nope h0 | nope h1 | roped qpe
    q_in = nc.dram_tensor("q_in", [NP, P, 3, TC], bf16)
    q_out = nc.dram_tensor("q_out", [NC_, P, 3, TC], bf16)


    with tile.TileContext(nc) as tc, nc.allow_low_precision(
        reason="bf16 matmul operands and outputs are intentional"
    ):
        with tc.tile_pool(name="persist", bufs=1) as pp:
            # persistent loads on the scalar queue (no waits -> no blocking)
            wkvbk_sb = pp.tile([P, NKV, HLOC * DN], bf16, name="wkvbk")
            nc.scalar.dma_start(out=wkvbk_sb, in_=wkvbk_d[:, :, :])
            wkvbv_sb = pp.tile([P, NKV, HLOC * DV], bf16, name="wkvbv")
            nc.scalar.dma_start(out=wkvbv_sb, in_=wkvbv_d[:, :, :])
            cosl_sb = pp.tile([P, TC], f32, name="cosl")
            nc.scalar.dma_start(out=cosl_sb, in_=cosl_d[:, :])
            sinl_sb = pp.tile([P, TC], f32, name="sinl")
            nc.scalar.dma_start(out=sinl_sb, in_=sinl_d[:, :])
            permr_sb = pp.tile([P, P], bf16, name="permr")
            nc.scalar.dma_start(out=permr_sb, in_=permr_d[:, :])
            # trimask/wo are not needed until stage B; their DMAs are
            # emitted there to keep the startup scalar queue short
            trimask_sb = pp.tile([P, P], bf16, name="trimask")
            wo_sb = pp.tile([P, HLOC, HID], bf16, name="wo")

            ones_f = pp.tile([P, P], f32, name="ones_f")
            nc.vector.memset(ones_f, 1.0)
            ones_sb = pp.tile([P, 1], f32r, name="ones")
            nc.vector.tensor_copy(ones_sb, ones_f[:, 0:1])
            ones_bf = pp.tile([P, 1], bf16, name="ones_bf")
            nc.vector.tensor_copy(ones_bf, ones_f[:, 0:1])
            col_ones = pp.tile([1, P], f32r, name="col_ones")
            nc.vector.tensor_copy(col_ones, ones_f[0:1, :])
            zmask = pp.tile([P, HLOC], f32, name="zmask")
            nc.vector.memset(zmask[0:DR, 0:1], 1.0)
            nc.vector.memset(zmask[DR:P, 0:1], 0.0)
            nc.vector.memset(zmask[0:DR, 1:2], 0.0)
            nc.vector.memset(zmask[DR:P, 1:2], 1.0)
            eps_sb = pp.tile([1, 1], f32, name="eps")
            nc.vector.memset(eps_sb, EPS)

            # h in two halves: coarse DMAs (the queue issues ~0.6us per
            # descriptor, so many small DMAs would gate startup)
            h_sb = [
                pp.tile([P, NKH // 2, TC], bf16, name=f"hh{i}") for i in range(2)
            ]
            nc.sync.dma_start(out=h_sb[0], in_=hT_d[:, 0 : NKH // 2, :])
            nc.sync.dma_start(out=h_sb[1], in_=hT_d[:, NKH // 2 : NKH, :])
            h_tiles = [h_sb[k // (NKH // 2)][:, k % (NKH // 2), :] for k in range(NKH)]

            pay_kv = pp.tile([P, NKV + 1, TC], bf16, name="paykv")
            qc_sb = pp.tile([P, NKQ, TC], bf16, name="qc")
            rq_b = pp.tile([P, TC], f32, name="rqb")
            rkv_b = pp.tile([P, TC], f32, name="rkvb")

            KT = [pp.tile([P, T], bf16, name=f"KT{h}") for h in range(HLOC)]
            kpe_raw = pp.tile([P, T], bf16, name="kperaw")
            kpe2 = [pp.tile([P, T], bf16, name=f"kpe2{h}") for h in range(HLOC)]
            kvn_sb = pp.tile([P, NKV, T], bf16, name="kvn")
            # post-AllToAll q readback, consumed directly by the matmuls
            payq2 = pp.tile([P, 3, NC_, TC], bf16, name="payq2")
            V_sb = [pp.tile([P, HLOC * DV], bf16, name=f"v{i}") for i in range(NKB)]

            # ---------------- Stage A: sharded projections ----------------
            with (
                tc.tile_pool(name="astream", bufs=3) as sp_,
                tc.tile_pool(name="aqbstream", bufs=3) as qbp,
                tc.tile_pool(name="aqpay", bufs=3) as qpay,
                tc.tile_pool(name="asmall", bufs=1) as smp,
                tc.tile_pool(name="aps", bufs=3, space="PSUM") as s1ps,
                tc.tile_pool(name="arope", bufs=1, space="PSUM") as rps,
                tc.tile_pool(name="ssqps", bufs=1, space="PSUM") as ssqps,
                tc.tile_pool(name="upps", bufs=3, space="PSUM") as upps,
            ):
                ssq2 = ssqps.tile([1, 2 * TC], f32, name="ssq2")
                ssq_kv = ssq2[:, 0:TC]
                ssq_q = ssq2[:, TC : 2 * TC]

                def rope_combine(ps, dst, scale_b):
                    """dst = (ps*cos + perm(ps)*sin) [* scale_b]; perm is the
                    signed neox partner permutation applied on the PE."""
                    xb = smp.tile([P, TC], bf16, name="ropexb")
                    nc.vector.tensor_copy(xb, ps)
                    rot_ps = upps.tile([P, TC], f32, name="up")
                    nc.tensor.matmul(
                        rot_ps, lhsT=permr_sb, rhs=xb, start=True, stop=True
                    )
                    t5 = smp.tile([P, TC], f32, name="ropet5")
                    t6 = smp.tile([P, TC], f32, name="ropet6")
                    nc.vector.tensor_mul(t5, ps, cosl_sb)
                    nc.vector.tensor_mul(t6, rot_ps, sinl_sb)
                    if scale_b is None:
                        nc.vector.tensor_add(dst, t5, t6)
                    else:
                        nc.vector.tensor_add(t5, t5, t6)
                        nc.vector.tensor_mul(dst, t5, scale_b)

                # --- kv path first (its payload gates CC#1) ---
                rope_ps = None
                for m in range(NKV + 1):
                    wk_sb = sp_.tile([P, NKH, P], bf16, name="wstream")
                    nc.sync.dma_start(out=wk_sb, in_=wkva_d[:, m, :, :])
                    if m < NKV:
                        ps = s1ps.tile([P, TC], f32, name="s1")
                    else:
                        ps = rps.tile([P, TC], f32, name="rope")
                        rope_ps = ps
                    for k in range(NKH):
                        nc.tensor.matmul(
                            ps,
                            lhsT=wk_sb[:, k, :],
                            rhs=h_tiles[k],
                            start=(k == 0),
                            stop=(k == NKH - 1),
                        )
                    if m < NKV:
                        nc.vector.tensor_copy(pay_kv[:, m, :], ps)
                        sq = smp.tile([P, TC], f32r, name="sq", bufs=1)
                        nc.scalar.square(sq, ps)
                        nc.tensor.matmul(
                            ssq_kv,
                            lhsT=ones_sb,
                            rhs=sq,
                            start=(m == 0),
                            stop=(m == NKV - 1),
                        )

                # rkv scale + broadcast
                rkv = smp.tile([1, TC], f32r, name="rkv")
                nc.scalar.activation(
                    rkv, ssq_kv, func=AF.Sqrt, bias=eps_sb, scale=1.0 / KVL
                )
                nc.vector.reciprocal(rkv, rkv)
                rkvb_ps = upps.tile([P, TC], f32, name="up")
                nc.tensor.matmul(rkvb_ps, lhsT=col_ones, rhs=rkv, start=True, stop=True)
                nc.vector.tensor_copy(rkv_b, rkvb_ps)
                # roped k_pe (dup rows, unnormalized), then normalize latent
                rope_combine(rope_ps, pay_kv[:, NKV, :], None)
                for m in range(NKV):
                    nc.vector.tensor_mul(pay_kv[:, m, :], pay_kv[:, m, :], rkv_b)
                nc.scalar.dma_start(out=kv_in[:, :, :], in_=pay_kv)
                nc.gpsimd.collective_compute(
                    "AllGather",
                    mybir.AluOpType.bypass,
                    replica_groups=GRP,
                    ins=[kv_in[:, :, :].opt()],
                    outs=[kv_out[:, :, :, :].opt()],
                )

                # --- q path stage-1 ---
                for m in range(NKQ):
                    wq_sb = sp_.tile([P, NKH, P], bf16, name="wstream")
                    nc.sync.dma_start(out=wq_sb, in_=wqa_d[:, m, :, :])
                    ps = s1ps.tile([P, TC], f32, name="s1")
                    for k in range(NKH):
                        nc.tensor.matmul(
                            ps,
                            lhsT=wq_sb[:, k, :],
                            rhs=h_tiles[k],
                            start=(k == 0),
                            stop=(k == NKH - 1),
                        )
                    nc.vector.tensor_copy(qc_sb[:, m, :], ps)
                    sq = smp.tile([P, TC], f32r, name="sq", bufs=1)
                    nc.scalar.square(sq, ps)
                    nc.tensor.matmul(
                        ssq_q,
                        lhsT=ones_sb,
                        rhs=sq,
                        start=(m == 0),
                        stop=(m == NKQ - 1),
                    )
                rq = smp.tile([1, TC], f32r, name="rq")
                nc.scalar.activation(
                    rq, ssq_q, func=AF.Sqrt, bias=eps_sb, scale=1.0 / QL
                )
                nc.vector.reciprocal(rq, rq)
                rqb_ps = upps.tile([P, TC], f32, name="up")
                nc.tensor.matmul(rqb_ps, lhsT=col_ones, rhs=rq, start=True, stop=True)
                nc.vector.tensor_copy(rq_b, rqb_ps)

                # --- q up-projection: own tokens, ALL head-pairs ---
                for p_ in range(NP):
                    wqbs = qbp.tile([P, 3, NKQ, P], bf16, name="wqbs")
                    for mo_ in range(3):
                        nc.sync.dma_start(
                            out=wqbs[:, mo_, :, :], in_=wqb_d[:, p_, mo_, :, :]
                        )
                    pay = qpay.tile([P, 3, TC], bf16, name="qpay")
                    for mo in range(2):
                        ps = upps.tile([P, TC], f32, name="up")
                        for k in range(NKQ):
                            nc.tensor.matmul(
                                ps,
                                lhsT=wqbs[:, mo, k, :],
                                rhs=qc_sb[:, k, :],
                                start=(k == 0),
                                stop=(k == NKQ - 1),
                            )
                        nc.vector.tensor_mul(pay[:, mo, :], ps, rq_b)
                    ps_r = upps.tile([P, TC], f32, name="up")
                    for k in range(NKQ):
                        nc.tensor.matmul(
                            ps_r,
                            lhsT=wqbs[:, 2, k, :],
                            rhs=qc_sb[:, k, :],
                            start=(k == 0),
                            stop=(k == NKQ - 1),
                        )
                    rope_combine(ps_r, pay[:, 2, :], rq_b)
                    nc.scalar.dma_start(out=q_in[p_, :, :, :], in_=pay)
                nc.gpsimd.collective_compute(
                    "AllToAll",
                    mybir.AluOpType.bypass,
                    replica_groups=GRP,
                    ins=[q_in[:, :, :, :].opt()],
                    outs=[q_out[:, :, :, :].opt()],
                )

            # ---------------- Stage B: gather-side compute ----------------
            with (
                tc.tile_pool(name="bpt", bufs=6) as ptp,
                tc.tile_pool(name="bsmall", bufs=3) as bsm,
                tc.tile_pool(name="blacc", bufs=2) as lap,
                tc.tile_pool(name="sps", bufs=3, space="PSUM") as spsp,
                tc.tile_pool(name="otps", bufs=2, space="PSUM") as otpsp,
            ):
                nc.scalar.dma_start(out=trimask_sb, in_=trimask_d[:, :])
                nc.scalar.dma_start(out=wo_sb, in_=wo_d[:, :, :])
                # kv readback + K/V up-projection for own heads
                # (wait floor stops the scheduler from emitting these
                # CC-gated triggers early enough to block the queues)
                with tc.tile_wait_until(0.115):
                    for r in range(NC_):
                        nc.sync.dma_start(
                            out=kvn_sb[:, :, r * TC : (r + 1) * TC],
                            in_=kv_out[r, :, 0:NKV, :],
                        )
                        nc.sync.dma_start(
                            out=kpe_raw[:, r * TC : (r + 1) * TC],
                            in_=kv_out[r, :, NKV, :],
                        )
                for h in range(HLOC):
                    nc.vector.tensor_scalar_mul(
                        kpe2[h], kpe_raw, zmask[:, h : h + 1]
                    )
                for h in range(HLOC):
                    for j in range(T // QC):
                        ps = otpsp.tile([P, QC], f32, name="otps")
                        for k in range(NKV):
                            nc.tensor.matmul(
                                ps,
                                lhsT=wkvbk_sb[:, k, h * P : (h + 1) * P],
                                rhs=kvn_sb[:, k, j * QC : (j + 1) * QC],
                                start=(k == 0),
                                stop=(k == NKV - 1),
                            )
                        nc.vector.tensor_copy(KT[h][:, j * QC : (j + 1) * QC], ps)
                for tt in range(NKB):
                    ps = otpsp.tile([P, QC], f32, name="otps")[:, : HLOC * DV]
                    for k in range(NKV):
                        nc.tensor.matmul(
                            ps,
                            lhsT=kvn_sb[:, k, tt * P : (tt + 1) * P],
                            rhs=wkvbv_sb[:, k, :],
                            start=(k == 0),
                            stop=(k == NKV - 1),
                        )
                    nc.vector.tensor_copy(V_sb[tt], ps)

                # q readback straight into the bf16 operand buffer
                with tc.tile_wait_until(0.150):
                    for r in range(NC_):
                        eng = nc.scalar if r % 2 else nc.sync
                        eng.dma_start(
                            out=payq2[:, :, r, :], in_=q_out[r, :, :, :]
                        )

                # ---------------- attention ----------------
                OT_sb = [
                    [ptp.tile([P, QC], bf16, name=f"ot{h}_{j}", bufs=1) for j in range(NQC)]
                    for h in range(HLOC)
                ]

                def flush_norm(pend):
                    p_ot, p_l, p_h, p_j = pend
                    # one ones-matmul reduces the DVE-accumulated l over keys
                    lrow_ps = spsp.tile([1, QC], f32, name="lrow", bufs=1)
                    nc.tensor.matmul(lrow_ps, lhsT=ones_sb, rhs=p_l, start=True, stop=True)
                    # 1/l as exp(-ln l) on the act engine: the DVE reciprocal
                    # instruction takes 3.3us and blocks the DVE queue
                    lg = bsm.tile([1, QC], f32, name="lg")
                    nc.scalar.activation(lg, lrow_ps, func=AF.Ln)
                    recl = bsm.tile([1, QC], f32r, name="recl")
                    nc.scalar.activation(recl, lg, func=AF.Exp, scale=-1.0)
                    lb_ps = spsp.tile([P, QC], f32, name="yps", bufs=2)
                    nc.tensor.matmul(lb_ps, lhsT=col_ones, rhs=recl, start=True, stop=True)
                    lb = bsm.tile([P, QC], f32, name="lb")
                    nc.vector.tensor_copy(lb, lb_ps)
                    nc.vector.tensor_mul(OT_sb[p_h][p_j], p_ot, lb)

                def o_proj_chunk(j):
                    for sub4 in range(4):
                        tt = j * 4 + sub4
                        sub = sub4 * P
                        for n in range(HID // QC):
                            y_ps = spsp.tile([P, QC], f32, name="yps", bufs=2)
                            for h in range(HLOC):
                                nc.tensor.matmul(
                                    y_ps,
                                    lhsT=OT_sb[h][j][:, sub : sub + P],
                                    rhs=wo_sb[:, h, n * QC : (n + 1) * QC],
                                    start=(h == 0),
                                    stop=(h == HLOC - 1),
                                )
                            y_sb = ptp.tile([P, QC], bf16, name="ysb")
                            nc.vector.tensor_copy(y_sb, y_ps)
                            nc.sync.dma_start(
                                out=y_d[tt * P : (tt + 1) * P, n * QC : (n + 1) * QC],
                                in_=y_sb,
                            )

                pend = None
                for j in range(NQC):
                    pend_oproj = j - 1 if j > 0 else None
                    for h in range(HLOC):
                        ot_ps = otpsp.tile([P, QC], f32, name="otps")
                        l_acc = lap.tile([P, QC], f32r, name="lacc")
                        nkb = 4 * (j + 1)
                        qcol0 = 2 * j
                        def pv_emit(pv):
                            pt_, cs_, ki_ = pv
                            nc.tensor.matmul(
                                ot_ps[:, cs_:],
                                lhsT=V_sb[ki_][:, h * DV : (h + 1) * DV],
                                rhs=pt_[:, cs_:],
                                start=(ki_ == 0),
                                stop=(ki_ == nkb - 1),
                            )
                            # on gpsimd: the DVE's 3.3us reciprocal must not
                            # delay these, the flush reduce waits on them
                            if ki_ == 0:
                                nc.gpsimd.tensor_copy(l_acc, pt_)
                            else:
                                nc.gpsimd.tensor_add(
                                    l_acc[:, cs_:], l_acc[:, cs_:], pt_[:, cs_:]
                                )

                        # software-pipelined: PV/l for ki trail the scores
                        # for ki+2, so the PE never sits behind the exp
                        pend_pvs = []
                        for ki in range(nkb):
                            s2 = spsp.tile([P, QC], f32, name="sps2")
                            nc.tensor.matmul(
                                s2,
                                lhsT=KT[h][:, ki * P : (ki + 1) * P],
                                rhs=payq2[:, h, qcol0 : qcol0 + 2, :],
                                start=True,
                                stop=False,
                            )
                            nc.tensor.matmul(
                                s2,
                                lhsT=kpe2[h][:, ki * P : (ki + 1) * P],
                                rhs=payq2[:, 2, qcol0 : qcol0 + 2, :],
                                start=False,
                                stop=True,
                            )
                            pt = ptp.tile([P, QC], bf16, name="pt")
                            nc.scalar.activation(pt, s2, func=AF.Exp, scale=SCALE)
                            diag = (ki // 4 == j)
                            cs = (ki % 4) * P if diag else 0
                            if diag:
                                nc.gpsimd.tensor_mul(
                                    pt[:, cs : cs + P],
                                    pt[:, cs : cs + P],
                                    trimask_sb,
                                )
                            pend_pvs.append((pt, cs, ki))
                            if len(pend_pvs) > 2:
                                pv_emit(pend_pvs.pop(0))
                            if ki == 2 and pend is not None:
                                flush_norm(pend)
                                pend = None
                            if ki == 5 and pend_oproj is not None:
                                # o_proj for j-1, deferred so the flush
                                # chain for its OT tiles has time to drain
                                o_proj_chunk(pend_oproj)
                                pend_oproj = None
                        for pv in pend_pvs:
                            pv_emit(pv)
                        pend = (ot_ps, l_acc, h, j)
                flush_norm(pend)
                o_proj_chunk(NQC - 1)
    _split_multi_waits(nc, mybir)
    return nc


def _host_prep(inputs):
    import ml_dtypes

    bf = ml_dtypes.bfloat16
    hs = np.ascontiguousarray(np.asarray(inputs["hidden_states"], np.float32))
    pos = np.asarray(inputs["positions"], np.int32)
    w_qa = np.asarray(inputs["w_qa"], np.float32)
    q_ln = np.asarray(inputs["q_a_ln_w"], np.float32)
    w_qb = np.asarray(inputs["w_qb"], np.float32)
    w_kva = np.asarray(inputs["w_kva"], np.float32)
    kv_ln = np.asarray(inputs["kv_a_ln_w"], np.float32)
    w_kvb = np.asarray(inputs["w_kvb"], np.float32)
    w_o = np.asarray(inputs["w_o"], np.float32)

    # a-projections, pre-tiled: [p, m, k, col]
    wqa_b = np.ascontiguousarray(
        w_qa.reshape(NKH, P, NKQ, P).transpose(1, 2, 0, 3)
    ).astype(bf)

    kva_lat = w_kva[:, :KVL]
    kva_rope = w_kva[:, KVL:]                      # [2048, 64]
    wkva_aug = np.concatenate(
        [kva_lat, kva_rope, kva_rope], axis=1
    )  # [2048, 512+128]
    wkva_b = np.ascontiguousarray(
        wkva_aug.reshape(NKH, P, NKV + 1, P).transpose(1, 2, 0, 3)
    ).astype(bf)

    # rope tables (dup-row) + the signed neox partner permutation
    inv_freq = (
        1.0 / (10000.0 ** (np.arange(0, DR, 2, dtype=np.float32) / DR))
    ).astype(np.float32)
    freqs = pos.astype(np.float32)[:, None] * inv_freq[None, :]  # [T, 32]
    emb = np.concatenate([freqs, freqs], axis=-1)  # [T, 64]
    cosT = np.cos(emb).T.astype(np.float32)        # [64, T]
    sinT = np.sin(emb).T.astype(np.float32)        # [64, T]
    cos2 = np.ascontiguousarray(np.concatenate([cosT, cosT], axis=0))  # [128, T]
    sin2 = np.ascontiguousarray(np.concatenate([sinT, sinT], axis=0))
    # rot = perm^T @ x: out[i] = -x[i+32] (i<32), x[i-32] (32<=i<64), per block
    permr = np.zeros((P, P), dtype=np.float32)
    hh = DR // 2
    for b0 in range(0, P, DR):
        for i in range(hh):
            permr[b0 + hh + i, b0 + i] = -1.0      # lhsT[src, dst]
            permr[b0 + i, b0 + hh + i] = 1.0
    permr = permr.astype(bf)

    # q b-projection, ALL head-pairs, ln folded: [p, pair, mo(3), k, col]
    w_qb_f = (w_qb * q_ln[:, None]).reshape(QL, H, DQK)
    blocks = []
    for p_ in range(NP):
        h0, h1 = 2 * p_, 2 * p_ + 1
        ropeA = np.concatenate(
            [w_qb_f[:, h0, DN:], w_qb_f[:, h1, DN:]], axis=1
        )  # [QL, 128]
        blocks.append(
            np.stack(
                [w_qb_f[:, h0, :DN], w_qb_f[:, h1, :DN], ropeA], axis=0
            )  # [3, QL, 128]
        )
    wqb_all = np.stack(blocks, axis=0)  # [NP, 3, QL, 128]
    wqb_aug = np.ascontiguousarray(
        wqb_all.reshape(NP, 3, NKQ, P, P).transpose(3, 0, 1, 2, 4)
    ).astype(bf)  # [p, pair, mo, k, col]

    w_kvb_f = (w_kvb * kv_ln[:, None]).reshape(KVL, H, DN + DV)
    w_o_r = w_o.reshape(H, DV, HID)
    trimask = np.triu(np.ones((P, P), dtype=np.float32)).astype(bf)

    per_core = []
    for i in range(NC_):
        hh = [HLOC * i + x for x in range(HLOC)]
        t0 = i * TC
        hT = np.ascontiguousarray(
            hs[t0 : t0 + TC].reshape(TC, NKH, P).transpose(2, 1, 0)
        ).astype(bf)
        wkvbk = np.ascontiguousarray(
            np.concatenate([w_kvb_f[:, h, :DN] for h in hh], axis=1)
            .reshape(NKV, P, HLOC * DN)
            .transpose(1, 0, 2)
        ).astype(bf)
        wkvbv = np.ascontiguousarray(
            np.concatenate([w_kvb_f[:, h, DN:] for h in hh], axis=1)
            .reshape(NKV, P, HLOC * DV)
            .transpose(1, 0, 2)
        ).astype(bf)
        wo_i = np.ascontiguousarray(
            np.stack([w_o_r[h] for h in hh], axis=0).transpose(1, 0, 2)
        ).astype(bf)  # [p, h, HID]
        per_core.append(
            dict(
                hT=hT,
                wqa=wqa_b,
                wkva=wkva_b,
                wqb=wqb_aug,
                wkvbk=wkvbk,
                wkvbv=wkvbv,
                wo=wo_i,
                cosl=np.ascontiguousarray(cos2[:, t0 : t0 + TC]),
                sinl=np.ascontiguousarray(sin2[:, t0 : t0 + TC]),
                permr=permr,
                trimask=trimask,
            )
        )
    return per_core


def kernel(**inputs):
    global LAST_RESULTS
    from concourse.bass_utils import run_bass_kernel_spmd

    if "nc" not in _CACHE:
        _CACHE["nc"] = _build_program()
    nc = _CACHE["nc"]

    in_maps = _host_prep(inputs)
    res = run_bass_kernel_spmd(nc, in_maps, core_ids=list(range(NC_)))
    LAST_RESULTS = res
    out = np.zeros((T, HID), dtype=np.float32)
    for r in res.results:
        out += np.asarray(r["y"], dtype=np.float32)
    return out


# revision 50
# speedup vs baseline: 1.0259x; 1.0259x over previous
"""DeepseekV2 MLA attention (T=2048, H=16) on 8 trn2 cores.

Sharding v3 (all-bf16, throttle-aware):
- Stage 1 (a-projections) sequence-sharded: each core computes q_c /
  kv latent / roped k_pe for its own 256 tokens.
- kv latent (+ roped k_pe) AllGather'd; q up-projection sequence-sharded
  (own tokens, ALL 16 heads, rms scale + neox rope folded pre-exchange),
  then an AllToAll redistributes q by head-pair (2 heads/core, full T).
- The neox rope partner (-x2,x1) is formed by a DVE partition shuffle of
  the single rope projection psum (sin table carries the sign), instead
  of projecting a second rotated weight copy: saves 1/4 of the q b-proj
  matmuls + weights, and 1 of 6 kv a-proj tiles.
- Everything on the PE is bf16 (measured: bf16@512-free == f32r speed,
  at half the SBUF/DMA traffic; the chip power-throttles under load so
  every byte moved costs PE speed too). PSUM accumulation stays f32.
- Softmax denominator off the PE: DVE accumulates exp tiles into l_acc
  and tree-reduces over partitions; only the 1/l broadcast uses a tiny
  f32r ones-matmul. Causal diag via 0/1 tri mask post-exp. No row-max
  (logits ~N(0,1)). o_proj partials summed on the host (y in bf16).
- No dummy collective: the TileContext prelude barrier absorbs the
  ~40us comm bring-up; the kv AllGather queues behind it harmlessly.
- DMA triggers spread by engine: sync = weight streams + h tiles,
  scalar = persists + readbacks, gpsimd = collectives + payload writes.
"""

import numpy as np

T = 2048
HID = 2048
H = 16
NC_ = 8
HLOC = H // NC_          # 2 heads per core
NP = H // HLOC           # 8 head-pairs
QL = 1536                # q lora
KVL = 512                # kv lora
DN = 128                 # nope dim
DR = 64                  # rope dim
DQK = DN + DR            # 192
DV = 128
EPS = 1e-6
SCALE = float(DQK) ** -0.5
P = 128
TC = T // NC_            # 256 tokens per core (stage-1 shard)
NKQ = QL // P            # 12
NKV = KVL // P           # 4
NKH = HID // P           # 16
QC = 512                 # attention q-chunk
NQC = T // QC
NKB = T // P             # key blocks

_CACHE = {}
LAST_RESULTS = None


def _split_multi_waits(nc, mybir):
    """Walrus embeds at most one sem/event wait per TPB instruction; hoist
    extra waits onto preceding same-engine NoOps (queue FIFO keeps order)."""
    n = 0
    for f in nc.m.functions:
        for bb in f.blocks:
            new = []
            for inst in bb.instructions:
                si = getattr(inst, "sync_info", None)
                if si is not None and len(si.on_wait) > 1:
                    waits = list(si.on_wait)
                    for i, wv in enumerate(waits[:-1]):
                        noop = mybir.InstNoOp(
                            name=f"{inst.name}-wsplit{i}",
                            engine=inst.engine,
                            ins=[],
                            outs=[],
                        )
                        noop.bass_nofuse = True
                        noop.sync_info = mybir.SyncInfo(on_wait=[wv], on_update=[])
                        new.append(noop)
                    inst.sync_info = mybir.SyncInfo(
                        on_wait=[waits[-1]], on_update=list(si.on_update)
                    )
                    n += 1
                new.append(inst)
            bb.instructions = new
    return n


def _build_program():
    import concourse.bass as bass
    import concourse.tile as tile
    from concourse import mybir

    f32 = mybir.dt.float32
    bf16 = mybir.dt.bfloat16
    f32r = mybir.dt.float32r
    AF = mybir.ActivationFunctionType
    GRP = [list(range(NC_))]

    nc = bass.Bass(num_devices=NC_)

    # ---- dram parameters (per-core values supplied by the host) ----
    hT_d = nc.declare_dram_parameter("hT", [P, NKH, TC], bf16, isOutput=False)
    wqa_d = nc.declare_dram_parameter("wqa", [P, NKQ, NKH, P], bf16, isOutput=False)
    # latent 512 | ropeA dup 128
    wkva_d = nc.declare_dram_parameter(
        "wkva", [P, NKV + 1, NKH, P], bf16, isOutput=False
    )
    # q b-projection for ALL head-pairs: [p, pair, mo, k, col]
    # mo: 0/1 = nope h0/h1, 2 = ropeA (h0|h1)  (ln folded)
    wqb_d = nc.declare_dram_parameter("wqb", [P, NP, 3, NKQ, P], bf16, isOutput=False)
    wkvbk_d = nc.declare_dram_parameter("wkvbk", [P, NKV, HLOC * DN], bf16, isOutput=False)
    wkvbv_d = nc.declare_dram_parameter("wkvbv", [P, NKV, HLOC * DV], bf16, isOutput=False)
    wo_d = nc.declare_dram_parameter("wo", [P, HLOC, HID], bf16, isOutput=False)
    cosl_d = nc.declare_dram_parameter("cosl", [P, TC], f32, isOutput=False)
    sinl_d = nc.declare_dram_parameter("sinl", [P, TC], f32, isOutput=False)
    # signed permutation forming the neox rope partner (-x2,x1) per 64-block
    permr_d = nc.declare_dram_parameter("permr", [P, P], bf16, isOutput=False)
    trimask_d = nc.declare_dram_parameter("trimask", [P, P], bf16, isOutput=False)
    y_d = nc.declare_dram_parameter("y", [T, HID], bf16, isOutput=True)

    # ---- dram bounce buffers for the collectives ----
    # kv payload: 4 latent tiles + 1 roped kpe (dup) tile, [p, m, t]
    kv_in = nc.dram_tensor("kv_in", [P, NKV + 1, TC], bf16)
    kv_out = nc.dram_tensor(
        "kv_out", [NC_, P, NKV + 1, TC], bf16, addr_space="Shared"
    )
    # q payload: per dst head-pair [p, mo(3), t]: nope h0 | nope h1 | roped qpe
    q_in = nc.dram_tensor("q_in", [NP, P, 3, TC], bf16)
    q_out = nc.dram_tensor("q_out", [NC_, P, 3, TC], bf16)


    with tile.TileContext(nc) as tc, nc.allow_low_precision(
        reason="bf16 matmul operands and outputs are intentional"
    ):
        with tc.tile_pool(name="persist", bufs=1) as pp:
            # persistent loads on the scalar queue (no waits -> no blocking)
            wkvbk_sb = pp.tile([P, NKV, HLOC * DN], bf16, name="wkvbk")
            nc.scalar.dma_start(out=wkvbk_sb, in_=wkvbk_d[:, :, :])
            wkvbv_sb = pp.tile([P, NKV, HLOC * DV], bf16, name="wkvbv")
            nc.scalar.dma_start(out=wkvbv_sb, in_=wkvbv_d[:, :, :])
            cosl_sb = pp.tile([P, TC], f32, name="cosl")
            nc.scalar.dma_start(out=cosl_sb, in_=cosl_d[:, :])
            sinl_sb = pp.tile([P, TC], f32, name="sinl")
            nc.scalar.dma_start(out=sinl_sb, in_=sinl_d[:, :])
            permr_sb = pp.tile([P, P], bf16, name="permr")
            nc.scalar.dma_start(out=permr_sb, in_=permr_d[:, :])
            # trimask/wo are not needed until stage B; their DMAs are
            # emitted there to keep the startup scalar queue short
            trimask_sb = pp.tile([P, P], bf16, name="trimask")
            wo_sb = pp.tile([P, HLOC, HID], bf16, name="wo")

            ones_f = pp.tile([P, P], f32, name="ones_f")
            nc.vector.memset(ones_f, 1.0)
            ones_sb = pp.tile([P, 1], f32r, name="ones")
            nc.vector.tensor_copy(ones_sb, ones_f[:, 0:1])
            ones_bf = pp.tile([P, 1], bf16, name="ones_bf")
            nc.vector.tensor_copy(ones_bf, ones_f[:, 0:1])
            col_ones = pp.tile([1, P], f32r, name="col_ones")
            nc.vector.tensor_copy(col_ones, ones_f[0:1, :])
            zmask = pp.tile([P, HLOC], f32, name="zmask")
            nc.vector.memset(zmask[0:DR, 0:1], 1.0)
            nc.vector.memset(zmask[DR:P, 0:1], 0.0)
            nc.vector.memset(zmask[0:DR, 1:2], 0.0)
            nc.vector.memset(zmask[DR:P, 1:2], 1.0)
            eps_sb = pp.tile([1, 1], f32, name="eps")
            nc.vector.memset(eps_sb, EPS)

            # h in two halves: coarse DMAs (the queue issues ~0.6us per
            # descriptor, so many small DMAs would gate startup)
            h_sb = [
                pp.tile([P, NKH // 2, TC], bf16, name=f"hh{i}") for i in range(2)
            ]
            nc.sync.dma_start(out=h_sb[0], in_=hT_d[:, 0 : NKH // 2, :])
            nc.sync.dma_start(out=h_sb[1], in_=hT_d[:, NKH // 2 : NKH, :])
            h_tiles = [h_sb[k // (NKH // 2)][:, k % (NKH // 2), :] for k in range(NKH)]

            pay_kv = pp.tile([P, NKV + 1, TC], bf16, name="paykv")
            qc_sb = pp.tile([P, NKQ, TC], bf16, name="qc")
            rq_b = pp.tile([P, TC], f32, name="rqb")
            rkv_b = pp.tile([P, TC], f32, name="rkvb")

            KT = [pp.tile([P, T], bf16, name=f"KT{h}") for h in range(HLOC)]
            kpe_raw = pp.tile([P, T], bf16, name="kperaw")
            kpe2 = [pp.tile([P, T], bf16, name=f"kpe2{h}") for h in range(HLOC)]
            kvn_sb = pp.tile([P, NKV, T], bf16, name="kvn")
            # post-AllToAll q readback, consumed directly by the matmuls
            payq2 = pp.tile([P, 3, NC_, TC], bf16, name="payq2")
            V_sb = [pp.tile([P, HLOC * DV], bf16, name=f"v{i}") for i in range(NKB)]

            # ---------------- Stage A: sharded projections ----------------
            with (
                tc.tile_pool(name="astream", bufs=3) as sp_,
                tc.tile_pool(name="aqbstream", bufs=3) as qbp,
                tc.tile_pool(name="aqpay", bufs=3) as qpay,
                tc.tile_pool(name="asmall", bufs=1) as smp,
                tc.tile_pool(name="aps", bufs=3, space="PSUM") as s1ps,
                tc.tile_pool(name="arope", bufs=1, space="PSUM") as rps,
                tc.tile_pool(name="ssqps", bufs=1, space="PSUM") as ssqps,
                tc.tile_pool(name="upps", bufs=3, space="PSUM") as upps,
            ):
                ssq2 = ssqps.tile([1, 2 * TC], f32, name="ssq2")
                ssq_kv = ssq2[:, 0:TC]
                ssq_q = ssq2[:, TC : 2 * TC]

                def rope_combine(ps, dst, scale_b):
                    """dst = (ps*cos + perm(ps)*sin) [* scale_b]; perm is the
                    signed neox partner permutation applied on the PE."""
                    xb = smp.tile([P, TC], bf16, name="ropexb")
                    nc.vector.tensor_copy(xb, ps)
                    rot_ps = upps.tile([P, TC], f32, name="up")
                    nc.tensor.matmul(
                        rot_ps, lhsT=permr_sb, rhs=xb, start=True, stop=True
                    )
                    t5 = smp.tile([P, TC], f32, name="ropet5")
                    t6 = smp.tile([P, TC], f32, name="ropet6")
                    nc.vector.tensor_mul(t5, ps, cosl_sb)
                    nc.vector.tensor_mul(t6, rot_ps, sinl_sb)
                    if scale_b is None:
                        nc.vector.tensor_add(dst, t5, t6)
                    else:
                        nc.vector.tensor_add(t5, t5, t6)
                        nc.vector.tensor_mul(dst, t5, scale_b)

                # --- kv path first (its payload gates CC#1) ---
                rope_ps = None
                for m in range(NKV + 1):
                    wk_sb = sp_.tile([P, NKH, P], bf16, name="wstream")
                    nc.sync.dma_start(out=wk_sb, in_=wkva_d[:, m, :, :])
                    if m < NKV:
                        ps = s1ps.tile([P, TC], f32, name="s1")
                    else:
                        ps = rps.tile([P, TC], f32, name="rope")
                        rope_ps = ps
                    for k in range(NKH):
                        nc.tensor.matmul(
                            ps,
                            lhsT=wk_sb[:, k, :],
                            rhs=h_tiles[k],
                            start=(k == 0),
                            stop=(k == NKH - 1),
                        )
                    if m < NKV:
                        nc.vector.tensor_copy(pay_kv[:, m, :], ps)
                        sq = smp.tile([P, TC], f32r, name="sq", bufs=1)
                        nc.scalar.square(sq, ps)
                        nc.tensor.matmul(
                            ssq_kv,
                            lhsT=ones_sb,
                            rhs=sq,
                            start=(m == 0),
                            stop=(m == NKV - 1),
                        )

                # rkv scale + broadcast
                rkv = smp.tile([1, TC], f32r, name="rkv")
                nc.scalar.activation(
                    rkv, ssq_kv, func=AF.Sqrt, bias=eps_sb, scale=1.0 / KVL
                )
                nc.vector.reciprocal(rkv, rkv)
                rkvb_ps = upps.tile([P, TC], f32, name="up")
                nc.tensor.matmul(rkvb_ps, lhsT=col_ones, rhs=rkv, start=True, stop=True)
                nc.vector.tensor_copy(rkv_b, rkvb_ps)
                # roped k_pe (dup rows, unnormalized), then normalize latent
                rope_combine(rope_ps, pay_kv[:, NKV, :], None)
                for m in range(NKV):
                    nc.vector.tensor_mul(pay_kv[:, m, :], pay_kv[:, m, :], rkv_b)
                nc.scalar.dma_start(out=kv_in[:, :, :], in_=pay_kv)
                nc.gpsimd.collective_compute(
                    "AllGather",
                    mybir.AluOpType.bypass,
                    replica_groups=GRP,
                    ins=[kv_in[:, :, :].opt()],
                    outs=[kv_out[:, :, :, :].opt()],
                )

                # --- q path stage-1 ---
                for m in range(NKQ):
                    wq_sb = sp_.tile([P, NKH, P], bf16, name="wstream")
                    nc.sync.dma_start(out=wq_sb, in_=wqa_d[:, m, :, :])
                    ps = s1ps.tile([P, TC], f32, name="s1")
                    for k in range(NKH):
                        nc.tensor.matmul(
                            ps,
                            lhsT=wq_sb[:, k, :],
                            rhs=h_tiles[k],
                            start=(k == 0),
                            stop=(k == NKH - 1),
                        )
                    nc.vector.tensor_copy(qc_sb[:, m, :], ps)
                    sq = smp.tile([P, TC], f32r, name="sq", bufs=1)
                    nc.scalar.square(sq, ps)
                    nc.tensor.matmul(
                        ssq_q,
                        lhsT=ones_sb,
                        rhs=sq,
                        start=(m == 0),
                        stop=(m == NKQ - 1),
                    )
                rq = smp.tile([1, TC], f32r, name="rq")
                nc.scalar.activation(
                    rq, ssq_q, func=AF.Sqrt, bias=eps_sb, scale=1.0 / QL
                )
                nc.vector.reciprocal(rq, rq)
                rqb_ps = upps.tile([P, TC], f32, name="up")
                nc.tensor.matmul(rqb_ps, lhsT=col_ones, rhs=rq, start=True, stop=True)
                nc.vector.tensor_copy(rq_b, rqb_ps)

                # --- q up-projection: own tokens, ALL head-pairs ---
                for p_ in range(NP):
                    wqbs = qbp.tile([P, 3, NKQ, P], bf16, name="wqbs")
                    for mo_ in range(3):
                        nc.sync.dma_start(
                            out=wqbs[:, mo_, :, :], in_=wqb_d[:, p_, mo_, :, :]
                        )
                    pay = qpay.tile([P, 3, TC], bf16, name="qpay")
                    for mo in range(2):
                        ps = upps.tile([P, TC], f32, name="up")
                        for k in range(NKQ):
                            nc.tensor.matmul(
                                ps,
                                lhsT=wqbs[:, mo, k, :],
                                rhs=qc_sb[:, k, :],
                                start=(k == 0),
                                stop=(k == NKQ - 1),
                            )
                        nc.vector.tensor_mul(pay[:, mo, :], ps, rq_b)
                    ps_r = upps.tile([P, TC], f32, name="up")
                    for k in range(NKQ):
                        nc.tensor.matmul(
                            ps_r,
                            lhsT=wqbs[:, 2, k, :],
                            rhs=qc_sb[:, k, :],
                            start=(k == 0),
                            stop=(k == NKQ - 1),
                        )
                    rope_combine(ps_r, pay[:, 2, :], rq_b)
                    nc.scalar.dma_start(out=q_in[p_, :, :, :], in_=pay)
                nc.gpsimd.collective_compute(
                    "AllToAll",
                    mybir.AluOpType.bypass,
                    replica_groups=GRP,
                    ins=[q_in[:, :, :, :].opt()],
                    outs=[q_out[:, :, :, :].opt()],
                )

            # ---------------- Stage B: gather-side compute ----------------
            with (
                tc.tile_pool(name="bpt", bufs=6) as ptp,
                tc.tile_pool(name="bsmall", bufs=3) as bsm,
                tc.tile_pool(name="sps", bufs=2, space="PSUM") as spsp,
                tc.tile_pool(name="otps", bufs=2, space="PSUM") as otpsp,
                tc.tile_pool(name="lps", bufs=2, space="PSUM") as lpsp,
            ):
                nc.scalar.dma_start(out=trimask_sb, in_=trimask_d[:, :])
                nc.scalar.dma_start(out=wo_sb, in_=wo_d[:, :, :])
                # kv readback + K/V up-projection for own heads
                # (wait floor stops the scheduler from emitting these
                # CC-gated triggers early enough to block the queues)
                with tc.tile_wait_until(0.115):
                    for r in range(NC_):
                        nc.sync.dma_start(
                            out=kvn_sb[:, :, r * TC : (r + 1) * TC],
                            in_=kv_out[r, :, 0:NKV, :],
                        )
                        nc.sync.dma_start(
                            out=kpe_raw[:, r * TC : (r + 1) * TC],
                            in_=kv_out[r, :, NKV, :],
                        )
                for h in range(HLOC):
                    nc.vector.tensor_scalar_mul(
                        kpe2[h], kpe_raw, zmask[:, h : h + 1]
                    )
                for h in range(HLOC):
                    for j in range(T // QC):
                        ps = otpsp.tile([P, QC], f32, name="otps")
                        for k in range(NKV):
                            nc.tensor.matmul(
                                ps,
                                lhsT=wkvbk_sb[:, k, h * P : (h + 1) * P],
                                rhs=kvn_sb[:, k, j * QC : (j + 1) * QC],
                                start=(k == 0),
                                stop=(k == NKV - 1),
                            )
                        nc.vector.tensor_copy(KT[h][:, j * QC : (j + 1) * QC], ps)
                for tt in range(NKB):
                    ps = otpsp.tile([P, QC], f32, name="otps")[:, : HLOC * DV]
                    for k in range(NKV):
                        nc.tensor.matmul(
                            ps,
                            lhsT=kvn_sb[:, k, tt * P : (tt + 1) * P],
                            rhs=wkvbv_sb[:, k, :],
                            start=(k == 0),
                            stop=(k == NKV - 1),
                        )
                    nc.vector.tensor_copy(V_sb[tt], ps)

                # q readback straight into the bf16 operand buffer
                with tc.tile_wait_until(0.150):
                    for r in range(NC_):
                        eng = nc.scalar if r % 2 else nc.sync
                        eng.dma_start(
                            out=payq2[:, :, r, :], in_=q_out[r, :, :, :]
                        )

                # ---------------- attention ----------------
                OT_sb = [
                    [ptp.tile([P, QC], bf16, name=f"ot{h}_{j}", bufs=1) for j in range(NQC)]
                    for h in range(HLOC)
                ]

                def flush_norm(pend):
                    p_ot, p_l, p_h, p_j = pend
                    # 1/l as exp(-ln l) on the act engine: the DVE reciprocal
                    # instruction takes 3.3us and blocks the DVE queue
                    lg = bsm.tile([1, QC], f32, name="lg")
                    nc.scalar.activation(lg, p_l, func=AF.Ln)
                    recl = bsm.tile([1, QC], f32r, name="recl")
                    nc.scalar.activation(recl, lg, func=AF.Exp, scale=-1.0)
                    lb_ps = spsp.tile([P, QC], f32, name="yps", bufs=2)
                    nc.tensor.matmul(lb_ps, lhsT=col_ones, rhs=recl, start=True, stop=True)
                    lb = bsm.tile([P, QC], f32, name="lb")
                    nc.vector.tensor_copy(lb, lb_ps)
                    nc.vector.tensor_mul(OT_sb[p_h][p_j], p_ot, lb)

                def o_proj_chunk(j):
                    for sub4 in range(4):
                        tt = j * 4 + sub4
                        sub = sub4 * P
                        for n in range(HID // QC):
                            y_ps = spsp.tile([P, QC], f32, name="yps", bufs=2)
                            for h in range(HLOC):
                                nc.tensor.matmul(
                                    y_ps,
                                    lhsT=OT_sb[h][j][:, sub : sub + P],
                                    rhs=wo_sb[:, h, n * QC : (n + 1) * QC],
                                    start=(h == 0),
                                    stop=(h == HLOC - 1),
                                )
                            y_sb = ptp.tile([P, QC], bf16, name="ysb")
                            if n % 2 == 0:
                                nc.vector.tensor_copy(y_sb, y_ps)
                            else:
                                nc.scalar.copy(y_sb, y_ps)
                            nc.sync.dma_start(
                                out=y_d[tt * P : (tt + 1) * P, n * QC : (n + 1) * QC],
                                in_=y_sb,
                            )

                pend = None
                for j in range(NQC):
                    pend_oproj = j - 1 if j > 0 else None
                    for h in range(HLOC):
                        ot_ps = otpsp.tile([P, QC], f32, name="otps")
                        l_ps = lpsp.tile([1, QC], f32, name="lps")
                        nkb = 4 * (j + 1)
                        qcol0 = 2 * j
                        def pv_emit(pv):
                            pt_, cs_, ki_ = pv
                            nc.tensor.matmul(
                                ot_ps[:, cs_:],
                                lhsT=V_sb[ki_][:, h * DV : (h + 1) * DV],
                                rhs=pt_[:, cs_:],
                                start=(ki_ == 0),
                                stop=(ki_ == nkb - 1),
                            )
                            nc.tensor.matmul(
                                l_ps[:, cs_:],
                                lhsT=ones_bf,
                                rhs=pt_[:, cs_:],
                                start=(ki_ == 0),
                                stop=(ki_ == nkb - 1),
                            )

                        # software-pipelined: PV/l for ki trail the scores
                        # for ki+2, so the PE never sits behind the exp
                        pend_pvs = []
                        for ki in range(nkb):
                            s2 = spsp.tile([P, QC], f32, name="sps2")
                            nc.tensor.matmul(
                                s2,
                                lhsT=KT[h][:, ki * P : (ki + 1) * P],
                                rhs=payq2[:, h, qcol0 : qcol0 + 2, :],
                                start=True,
                                stop=False,
                            )
                            nc.tensor.matmul(
                                s2,
                                lhsT=kpe2[h][:, ki * P : (ki + 1) * P],
                                rhs=payq2[:, 2, qcol0 : qcol0 + 2, :],
                                start=False,
                                stop=True,
                            )
                            pt = ptp.tile([P, QC], bf16, name="pt")
                            nc.scalar.activation(pt, s2, func=AF.Exp, scale=SCALE)
                            diag = (ki // 4 == j)
                            cs = (ki % 4) * P if diag else 0
                            if diag:
                                nc.gpsimd.tensor_mul(
                                    pt[:, cs : cs + P],
                                    pt[:, cs : cs + P],
                                    trimask_sb,
                                )
                            pend_pvs.append((pt, cs, ki))
                            if len(pend_pvs) > 2:
                                pv_emit(pend_pvs.pop(0))
                            if ki == 2 and pend is not None:
                                flush_norm(pend)
                                pend = None
                            if ki == 5 and pend_oproj is not None:
                                # o_proj for j-1, deferred so the flush
                                # chain for its OT tiles has time to drain
                                o_proj_chunk(pend_oproj)
                                pend_oproj = None
                        for pv in pend_pvs:
                            pv_emit(pv)
                        pend = (ot_ps, l_ps, h, j)
                flush_norm(pend)
                o_proj_chunk(NQC - 1)
    _split_multi_waits(nc, mybir)
    return nc


def _host_prep(inputs):
    import ml_dtypes

    bf = ml_dtypes.bfloat16
    hs = np.ascontiguousarray(np.asarray(inputs["hidden_states"], np.float32))
    pos = np.asarray(inputs["positions"], np.int32)
    w_qa = np.asarray(inputs["w_qa"], np.float32)
    q_ln = np.asarray(inputs["q_a_ln_w"], np.float32)
    w_qb = np.asarray(inputs["w_qb"], np.float32)
    w_kva = np.asarray(inputs["w_kva"], np.float32)
    kv_ln = np.asarray(inputs["kv_a_ln_w"], np.float32)
    w_kvb = np.asarray(inputs["w_kvb"], np.float32)
    w_o = np.asarray(inputs["w_o"], np.float32)

    # a-projections, pre-tiled: [p, m, k, col]
    wqa_b = np.ascontiguousarray(
        w_qa.reshape(NKH, P, NKQ, P).transpose(1, 2, 0, 3)
    ).astype(bf)

    kva_lat = w_kva[:, :KVL]
    kva_rope = w_kva[:, KVL:]                      # [2048, 64]
    wkva_aug = np.concatenate(
        [kva_lat, kva_rope, kva_rope], axis=1
    )  # [2048, 512+128]
    wkva_b = np.ascontiguousarray(
        wkva_aug.reshape(NKH, P, NKV + 1, P).transpose(1, 2, 0, 3)
    ).astype(bf)

    # rope tables (dup-row) + the signed neox partner permutation
    inv_freq = (
        1.0 / (10000.0 ** (np.arange(0, DR, 2, dtype=np.float32) / DR))
    ).astype(np.float32)
    freqs = pos.astype(np.float32)[:, None] * inv_freq[None, :]  # [T, 32]
    emb = np.concatenate([freqs, freqs], axis=-1)  # [T, 64]
    cosT = np.cos(emb).T.astype(np.float32)        # [64, T]
    sinT = np.sin(emb).T.astype(np.float32)        # [64, T]
    cos2 = np.ascontiguousarray(np.concatenate([cosT, cosT], axis=0))  # [128, T]
    sin2 = np.ascontiguousarray(np.concatenate([sinT, sinT], axis=0))
    # rot = perm^T @ x: out[i] = -x[i+32] (i<32), x[i-32] (32<=i<64), per block
    permr = np.zeros((P, P), dtype=np.float32)
    hh = DR // 2
    for b0 in range(0, P, DR):
        for i in range(hh):
            permr[b0 + hh + i, b0 + i] = -1.0      # lhsT[src, dst]
            permr[b0 + i, b0 + hh + i] = 1.0
    permr = permr.astype(bf)

    # q b-projection, ALL head-pairs, ln folded: [p, pair, mo(3), k, col]
    w_qb_f = (w_qb * q_ln[:, None]).reshape(QL, H, DQK)
    blocks = []
    for p_ in range(NP):
        h0, h1 = 2 * p_, 2 * p_ + 1
        ropeA = np.concatenate(
            [w_qb_f[:, h0, DN:], w_qb_f[:, h1, DN:]], axis=1
        )  # [QL, 128]
        blocks.append(
            np.stack(
                [w_qb_f[:, h0, :DN], w_qb_f[:, h1, :DN], ropeA], axis=0
            )  # [3, QL, 128]
        )
    wqb_all = np.stack(blocks, axis=0)  # [NP, 3, QL, 128]
    wqb_aug = np.ascontiguousarray(
        wqb_all.reshape(NP, 3, NKQ, P, P).transpose(3, 0, 1, 2, 4)
    ).astype(bf)  # [p, pair, mo, k, col]

    w_kvb_f = (w_kvb * kv_ln[:, None]).reshape(KVL, H, DN + DV)
    w_o_r = w_o.reshape(H, DV, HID)
    trimask = np.triu(np.ones((P, P), dtype=np.float32)).astype(bf)

    per_core = []
    for i in range(NC_):
        hh = [HLOC * i + x for x in range(HLOC)]
        t0 = i * TC
        hT = np.ascontiguousarray(
            hs[t0 : t0 + TC].reshape(TC, NKH, P).transpose(2, 1, 0)
        ).astype(bf)
        wkvbk = np.ascontiguousarray(
            np.concatenate([w_kvb_f[:, h, :DN] for h in hh], axis=1)
            .reshape(NKV, P, HLOC * DN)
            .transpose(1, 0, 2)
        ).astype(bf)
        wkvbv = np.ascontiguousarray(
            np.concatenate([w_kvb_f[:, h, DN:] for h in hh], axis=1)
            .reshape(NKV, P, HLOC * DV)
            .transpose(1, 0, 2)
        ).astype(bf)
        wo_i = np.ascontiguousarray(
            np.stack([w_o_r[h] for h in hh], axis=0).transpose(1, 0, 2)
        ).astype(bf)  # [p, h, HID]
        per_core.append(
            dict(
                hT=hT,
                wqa=wqa_b,
                wkva=wkva_b,
                wqb=wqb_aug,
                wkvbk=wkvbk,
                wkvbv=wkvbv,
                wo=wo_i,
                cosl=np.ascontiguousarray(cos2[:, t0 : t0 + TC]),
                sinl=np.ascontiguousarray(sin2[:, t0 : t0 + TC]),
                permr=permr,
                trimask=trimask,
            )
        )
    return per_core


def kernel(**inputs):
    global LAST_RESULTS
    from concourse.bass_utils import run_bass_kernel_spmd

    if "nc" not in _CACHE:
        _CACHE["nc"] = _build_program()
    nc = _CACHE["nc"]

    in_maps = _host_prep(inputs)
    res = run_bass_kernel_spmd(nc, in_maps, core_ids=list(range(NC_)))
    LAST_RESULTS = res
    out = np.zeros((T, HID), dtype=np.float32)
    for r in res.results:
        out += np.asarray(r["y"], dtype=np.float32)
    return out
